# revision 48
# baseline (speedup 1.0000x reference)
"""MoE (top-2 of 8 experts) Trainium2 kernel, expert-parallel across 8 NeuronCores.

Strategy (pure-GEMM device kernel, ~264us vs 548us baseline):
  - Host: gate (fp32, exact top-2 routing), then per expert pre-gather the
    routed token rows of x, transpose to [d, t], pad to CAP columns, and
    convert to fp16. Weights re-laid-out per expert for weight-stationary
    matmuls ([contraction, 128] lhsT tiles, contiguous per-tile DMA).
  - Device (per core = one expert): two dense GEMM phases, fp16 operands,
    fp32 PSUM accumulation:
      mm1: hT[f, t] = gelu(W1.T @ xT + b1)   (w1 stationary, xT moving)
      mm2: y[d, t]  = W2.T @ hT              (w2 stationary, hT moving)
    No gathers, scatters, transposes, or collectives on device. Tokens are
    processed in column chunks (512, 512, 66) so each matmul output fits one
    PSUM bank; chunk psum tags double-buffer across both phases. mm1 runs
    chunk-major over all 32 ftiles with w1 fully SBUF-resident, so the
    startup-critical DMA set is just b1 + the chunk-a block of xT + the
    first w1 slabs; PE warmup matmuls on zeros bridge the DMA ramp with the
    HAM activity window open, so the real matmul stream runs at the warm
    2.4 GHz roofline (512-col matmuls at ~218ns spacing) end to end.
  - Host: out[toks_e] += w_e * y_e.T per core, plus the (combine-weight @ b2)
    term; this is the unshard/combine step of the expert-parallel sharding.

Only the top-2 experts per token are ever computed (masked terms of the
reference are exactly zero), cutting FLOPs 4x vs the dense formulation.
"""

import math
import sys

for _p in ("/opt/trn_rl_repo", "/root/.axon_site/_ro/trn_rl_repo"):
    if _p not in sys.path:
        sys.path.append(_p)

import numpy as np

from contextlib import ExitStack

import concourse.bass as bass
import concourse.mybir as mybir
import concourse.tile as tile
from concourse import bacc
from concourse.bass_utils import run_bass_kernel_spmd

# Problem shapes (nn_MixtureOfExperts_45243185496830)
B, S, D, E, TOPK = 2, 2048, 1024, 8, 2
DFF = 4 * D
T = B * S            # 4096 tokens
P = 128
NCORES = 8

# Per-core token capacity. Routing is deterministic (fixed seed); max expert
# load is 1090. Tokens beyond CAP (should never happen) spill to a host-side
# exact-FFN fallback, so a load change degrades speed, not correctness.
CAP = 1090
CHUNKS = ((0, 512), (512, 1024), (1024, CAP))  # per-bank psum column chunks
HSPLIT = CHUNKS[0][1]                          # xta/xtb column split

F32 = mybir.dt.float32
F16 = mybir.dt.float16


def build_model():
    nc = bacc.Bacc(None, target_bir_lowering=False)

    # [d_in, dt, t] in two column blocks (chunk-a cols, chunk-b/t cols) so
    # each ships as one large fully-contiguous DMA
    HA = CHUNKS[0][1]
    xta_ext = nc.declare_dram_parameter("xta", [P, D // P, HA], F16, isOutput=False)
    xtb_ext = nc.declare_dram_parameter(
        "xtb", [P, D // P, CAP - HA], F16, isOutput=False
    )
    # [ft, d_in, dt, f_in]
    w1_ext = nc.declare_dram_parameter(
        "w1", [DFF // P, P, D // P, P], F16, isOutput=False
    )
    b1_ext = nc.declare_dram_parameter("b1", [P, DFF // P], F32, isOutput=False)
    # [dt, f_in, ft, d_in]
    w2_ext = nc.declare_dram_parameter(
        "w2", [D // P, P, DFF // P, P], F16, isOutput=False
    )
    out_ext = nc.declare_dram_parameter("out", [D // P, P, CAP], F16, isOutput=True)

    with tile.TileContext(nc) as tc, ExitStack() as ctx:
        const = ctx.enter_context(tc.tile_pool(name="const", bufs=1))
        xpool = ctx.enter_context(tc.tile_pool(name="xp", bufs=1))
        hpool = ctx.enter_context(tc.tile_pool(name="hp", bufs=1))
        w1pool = ctx.enter_context(tc.tile_pool(name="w1p", bufs=1))
        w2pool = ctx.enter_context(tc.tile_pool(name="w2p", bufs=2))
        ypool = ctx.enter_context(tc.tile_pool(name="yp", bufs=2))
        # psum tags are shared between mm1 and mm2 so the rotation double-
        # buffers both phases out of the same 6 banks
        ps = ctx.enter_context(tc.tile_pool(name="ps", bufs=2, space="PSUM"))

        # ---- input DMAs, split across all three rings in demand order ----
        # w1 stays fully SBUF-resident (64KB/partition): each slab is DMA'd
        # once and read by all three column-chunk passes
        w1_sb = w1pool.tile([P, DFF // P, D // P, P], F16, name="w1_sb")
        xta_sb = xpool.tile([P, D // P, HA], F16, name="xta_sb")
        xtb_sb = xpool.tile([P, D // P, CAP - HA], F16, name="xtb_sb")
        b1_sb = const.tile([P, DFF // P], F32, name="b1_sb")
        # STARTUP-CRITICAL set: b1, the chunk-a column block of xt (two large
        # contiguous transfers on the fast scalar ring), w1 slabs 0-2.
        # Everything else (chunk-b/t block, later w1 slabs, w2) queues BEHIND
        # on the same rings so it cannot steal HBM bandwidth from the
        # critical window. mm1 runs chunk-major over all ftiles, so the
        # whole first pass needs only the chunk-a block.
        nc.scalar.dma_start(xta_sb[:, :4], xta_ext[:, :4])
        nc.sync.dma_start(xta_sb[:, 4:], xta_ext[:, 4:])
        for ft in range(6):
            nc.scalar.dma_start(w1_sb[:, ft], w1_ext[ft])
        nc.gpsimd.dma_start(b1_sb, b1_ext[:])

        # ---- PE warmup: dummy matmuls on zeros so the HAM activity window
        # opens during the preamble and real matmuls run at 2.4 GHz from the
        # first instruction; they bridge until the critical DMA set has
        # landed (~11us) so the real stream then runs warm and gap-free
        warm_sb = const.tile([P, 512], F16, name="warm_sb")
        nc.vector.memset(warm_sb, 0.0)
        psw = ps.tile([P, 512], F32, tag="psw", name="psw", bufs=1)
        for _ in range(20):
            nc.tensor.matmul(psw[:, :], lhsT=warm_sb[:, :P], rhs=warm_sb[:, :],
                             start=True, stop=True)

        # ---- mm1: hT[f_in, ft, t] = gelu(W1.T @ xT + b1), fp16 ----
        w2t_pre = []
        hT = hpool.tile([P, DFF // P, CAP], F16, name="hT")
        for ci, (c0, c1) in enumerate(CHUNKS):
            for ft in range(DFF // P):
                # remaining w1 slabs stream on scalar, two ftiles ahead;
                # in-loop triggers there are naturally activation-paced
                nxt = ft + 2
                if ci == 0 and 6 <= nxt < DFF // P:
                    nc.scalar.dma_start(w1_sb[:, nxt], w1_ext[nxt])
                ps_ = ps.tile([P, c1 - c0], F32, tag=f"ps{ci}", name=f"ps{ci}")
                rhs_sb = xta_sb if ci == 0 else xtb_sb
                r0, r1 = (0, HA) if ci == 0 else (c0 - HA, c1 - HA)
                for dt in range(D // P):
                    nc.tensor.matmul(
                        ps_[:, :],
                        lhsT=w1_sb[:, ft, dt, :],
                        rhs=rhs_sb[:, dt, r0:r1],
                        start=(dt == 0),
                        stop=(dt == D // P - 1),
                    )
                nc.scalar.activation(
                    out=hT[:, ft, c0:c1],
                    in_=ps_[:, :],
                    func=mybir.ActivationFunctionType.Gelu,
                    bias=b1_sb[:, ft : ft + 1],
                    scale=1.0,
                )
                if ci == 0 and ft < 3:
                    # ramp fillers: keep the HAM activity window busy while
                    # the early ftiles run DMA-paced; free when DMA is the
                    # constraint, ~0.4us total otherwise
                    for _ in range(2):
                        nc.tensor.matmul(psw[:, :], lhsT=warm_sb[:, :P],
                                         rhs=warm_sb[:, :], start=True,
                                         stop=True)
                if ci == 0 and ft == 26:
                    # gate the chunk-b/t block and the first two w2 slabs on
                    # phase-a reaching ftile 26: the throwaway copy below
                    # gives their DMA triggers a WAR dependency, so these
                    # ~3.3MB flow in the tail of phase-a (when the w1 slab
                    # stream is ending) instead of starving it early on
                    nc.vector.tensor_copy(
                        out=xtb_sb[:, 0, :256], in_=hT[:, 26, :256]
                    )
                    nc.sync.dma_start(xtb_sb[:], xtb_ext[:])
                    for dt in range(2):
                        w2t = w2pool.tile([P, DFF // P, P], F16, tag="w2t",
                                          name="w2t")
                        nc.sync.dma_start(w2t, w2_ext[dt])
                        w2t_pre.append(w2t)

        # ---- mm2: y[d_in, t] = W2.T @ hT, accumulated over all 32 ftiles ----
        for dt in range(D // P):
            w2t = (
                w2t_pre[dt]
                if dt < 2
                else w2pool.tile([P, DFF // P, P], F16, tag="w2t", name="w2t")
            )
            if dt >= 2:
                nc.scalar.dma_start(w2t, w2_ext[dt])
            pss = [
                ps.tile([P, c1 - c0], F32, tag=f"ps{ci}", name=f"ps{ci}")
                for ci, (c0, c1) in enumerate(CHUNKS)
            ]
            for ft in range(DFF // P):
                for ci, (c0, c1) in enumerate(CHUNKS):
                    nc.tensor.matmul(
                        pss[ci][:, :],
                        lhsT=w2t[:, ft, :],
                        rhs=hT[:, ft, c0:c1],
                        start=(ft == 0),
                        stop=(ft == DFF // P - 1),
                    )
            y = ypool.tile([P, CAP], F16, tag="y", name="y")
            if dt == D // P - 1:
                # last dtile: split the psum->sbuf copies across DVE and ACT
                # and ship on two rings so the tail is as short as possible
                c0, c1 = CHUNKS[0]
                nc.vector.tensor_copy(out=y[:, c0:c1], in_=pss[0][:, :])
                nc.sync.dma_start(out_ext[dt][:, c0:c1], y[:, c0:c1])
                b0, b1_ = CHUNKS[1]
                nc.scalar.activation(
                    out=y[:, b0:b1_], in_=pss[1][:, :],
                    func=mybir.ActivationFunctionType.Copy,
                )
                nc.scalar.dma_start(out_ext[dt][:, b0:b1_], y[:, b0:b1_])
                t0, t1 = CHUNKS[2]
                nc.vector.tensor_copy(out=y[:, t0:t1], in_=pss[2][:, :])
                nc.sync.dma_start(out_ext[dt][:, t0:t1], y[:, t0:t1])
            else:
                for ci, (c0, c1) in enumerate(CHUNKS):
                    nc.vector.tensor_copy(out=y[:, c0:c1], in_=pss[ci][:, :])
                nc.sync.dma_start(out_ext[dt], y[:])

    nc.compile()
    return nc


_NC = None

# test harness hooks: set TRACE=True before calling kernel() to capture an
# NTFF profile; the BassKernelResults lands in LAST_RESULTS.
TRACE = False
LAST_RESULTS = None


def _get_model():
    global _NC
    if _NC is None:
        _NC = build_model()
    return _NC


def _route(x2, Wg, bg):
    """Host-side gate: exact fp32 top-2 routing (matches jax.lax.top_k)."""
    logits = x2 @ Wg + bg                      # [T, E] fp32
    order = np.argsort(-logits, axis=1, kind="stable")  # top_k tie-break: first idx
    i1, i2 = order[:, 0], order[:, 1]
    l1 = logits[np.arange(T), i1]
    l2 = logits[np.arange(T), i2]
    # softmax over the two selected logits (computed in f64, cast back)
    z = np.exp(np.float64(l2) - np.float64(l1))
    w1 = (1.0 / (1.0 + z)).astype(np.float32)
    w2 = (z / (1.0 + z)).astype(np.float32)
    return i1, i2, w1, w2


def make_in_maps(x2, W1, b1, W2, b2, Wg, bg):
    i1, i2, w1, w2 = _route(x2, Wg, bg)
    in_maps, metas = [], []
    for e in range(NCORES):
        sel1 = i1 == e
        sel2 = i2 == e
        toks = np.nonzero(sel1 | sel2)[0]
        wts = np.where(sel1[toks], w1[toks], w2[toks]).astype(np.float32)
        spill = toks[CAP:]
        toks = toks[:CAP]
        cnt = toks.shape[0]
        xg = np.zeros((CAP, D), np.float16)
        xg[:cnt] = x2[toks]
        xt = xg.T.reshape(D // P, P, CAP).transpose(1, 0, 2)
        m = {
            "xta": np.ascontiguousarray(xt[:, :, :HSPLIT]),
            "xtb": np.ascontiguousarray(xt[:, :, HSPLIT:]),
            "w1": np.ascontiguousarray(
                W1[e].reshape(D // P, P, DFF // P, P)
                .transpose(2, 1, 0, 3)
                .astype(np.float16)
            ),
            "b1": np.ascontiguousarray(b1[e].reshape(DFF // P, P).T),
            "w2": np.ascontiguousarray(
                W2[e].reshape(DFF // P, P, D // P, P)
                .transpose(2, 1, 0, 3)
                .astype(np.float16)
            ),
        }
        in_maps.append(m)
        metas.append((toks, wts, spill))
    # dense combine weights for the b2 term
    wdense = np.zeros((T, E), np.float32)
    ar = np.arange(T)
    wdense[ar, i1] = w1
    wdense[ar, i2] = w2
    return in_maps, metas, wdense


_erf = np.vectorize(math.erf)


def _host_ffn(x, W1e, b1e, W2e):
    """Exact-FFN fallback for tokens beyond CAP (normally never used)."""
    h = x.astype(np.float64) @ W1e.astype(np.float64) + b1e.astype(np.float64)
    h = 0.5 * h * (1.0 + _erf(h / np.sqrt(2.0)))
    return h @ W2e.astype(np.float64)


def kernel(x, W1, b1, W2, b2, Wg, bg):
    x = np.ascontiguousarray(np.asarray(x, dtype=np.float32))
    W1 = np.ascontiguousarray(np.asarray(W1, dtype=np.float32))
    b1 = np.ascontiguousarray(np.asarray(b1, dtype=np.float32))
    W2 = np.ascontiguousarray(np.asarray(W2, dtype=np.float32))
    b2 = np.ascontiguousarray(np.asarray(b2, dtype=np.float32))
    Wg = np.asarray(Wg, dtype=np.float32)
    bg = np.asarray(bg, dtype=np.float32)

    x2 = x.reshape(T, D)
    in_maps, metas, wdense = make_in_maps(x2, W1, b1, W2, b2, Wg, bg)

    nc = _get_model()
    global LAST_RESULTS
    res = run_bass_kernel_spmd(
        nc, in_maps, core_ids=list(range(NCORES)), trace=TRACE
    )
    LAST_RESULTS = res

    out = (wdense @ b2).astype(np.float32)             # [T, D] b2 term
    for e in range(NCORES):
        toks, wts, spill = metas[e]
        cnt = toks.shape[0]
        y = res.results[e]["out"].reshape(D, CAP)      # [d, t] fp16
        out[toks] += wts[:cnt, None] * y[:, :cnt].T.astype(np.float32)
        if spill.size:
            ys = _host_ffn(x2[spill], W1[e], b1[e], W2[e])
            out[spill] += wts[cnt:, None] * ys.astype(np.float32)
    return out.reshape(B, S, D)


if __name__ == "__main__":
    build_model()
    print("model built ok")
